# revision 19
# baseline (speedup 1.0000x reference)
"""Causal Conv1d (K=4) + bias + silu for TRN2, sharded over 8 NeuronCores.

Reference op: x (B=4, C_IN=2048, S=4096) fp32, weight (C_OUT=2048, C_IN, 4),
bias (C_OUT,);  out = silu(causal_conv1d(x, weight) + bias).

Sharding: data-parallel over sequence. Core c computes out[:, :, c*512:(c+1)*512]
from x[:, :, c*512-3 : c*512+512] (zero-padded left halo), full weight/bias.

Per-core compute: the conv is 16 k-tiles x 4 taps = 64 accumulating
128x128x512 matmuls per (batch, m-tile) PSUM group. Tap t uses a shifted
free-dim view of the resident x tile - no shifted copies are materialized.
Matmuls accumulate in fp32 PSUM. Weights are pre-transposed on the host to
(mi, p, ki, t, f) so each per-m-tile weight chunk is one DMA with
long-contiguous per-partition lines. The 4 batches accumulate into 4 PSUM
banks in parallel so each weight tile's uses are back-to-back and the next
m-tile (4 more banks) overlaps eviction (silu+bias on ScalarE) and store.
"""

import numpy as np

import concourse.bacc as bacc
import concourse.bass as bass
import concourse.mybir as mybir
import concourse.tile as tile
from concourse.bass_utils import run_bass_kernel_spmd

P = 128

# Problem constants (hardcoded per harness contract).
B = 4
C_IN = 2048
C_OUT = 2048
KTAPS = 4
S = 4096
N_CORES = 8
S_CHUNK = S // N_CORES          # 512
HALO = KTAPS - 1                # 3

# Matmul operand dtype: "f32r" (FP22-truncated fp32, rel err ~1.6e-4) or
# "f16" (fp16 operands, rel err ~1e-3, faster weight path via FWL).
MM_DTYPE = "f16"


def build_conv_nc(
    b, n_ki, n_mi, ktaps, s_chunk, ki_per_wchunk, act_fn=None, reps=1,
    mm_dtype=None,
):
    """Build the per-core Bass program.

    b:        batches
    n_ki:     C_IN / 128 contraction tiles
    n_mi:     C_OUT / 128 output tiles
    ktaps:    conv taps
    s_chunk:  output sequence columns per core
    ki_per_wchunk: k-tiles per weight DMA chunk
    """
    halo = ktaps - 1
    s_in = s_chunk + halo
    assert n_ki % ki_per_wchunk == 0
    n_wchunks = n_ki // ki_per_wchunk
    wchunk_cols = ki_per_wchunk * ktaps * P

    mm_dtype = mm_dtype or MM_DTYPE
    if mm_dtype == "f32r":
        in_dt = mybir.dt.float32      # dram dtype of x/w
        mm_dt = mybir.dt.float32r     # sbuf tile dtype fed to the PE
        cast_dma = False              # fp32->fp32r is a bitcast
    elif mm_dtype == "f16":
        in_dt = mybir.dt.float16      # host pre-converts x/w to fp16
        mm_dt = mybir.dt.float16
        cast_dma = False
    else:
        raise ValueError(mm_dtype)

    # Bacc (not raw Bass): its compile() splits multi-wait instructions into
    # event-semaphore sequences and moves matmul waits onto ldweights —
    # without it, walrus rejects any instruction carrying >1 sync wait.
    nc = bacc.Bacc("TRN2", target_bir_lowering=False, debug=False)

    x_d = nc.dram_tensor(
        "x", [b, n_ki * P, s_in], in_dt, kind="ExternalInput"
    ).ap()
    # weight pre-layout: (mi, p, ki, t, f) = W[mi*128+f, ki*128+p, t]
    w_d = nc.dram_tensor(
        "w", [n_mi, P, n_ki, ktaps, P], in_dt, kind="ExternalInput"
    ).ap()
    # bias pre-layout: [p, mi] = bias[mi*128+p]
    bias_d = nc.dram_tensor(
        "bias", [P, n_mi], mybir.dt.float32, kind="ExternalInput"
    ).ap()
    out_d = nc.dram_tensor(
        "out", [b, n_mi * P, s_chunk], mybir.dt.float32, kind="ExternalOutput"
    ).ap()

    f32 = mybir.dt.float32
    silu = act_fn if act_fn is not None else mybir.ActivationFunctionType.Silu

    def dma_src(ap):
        # fp32r tiles must be *written* as fp32r (BIR verifier rule);
        # the DRAM side carries the same bits, so bitcast the source.
        return ap.bitcast(mm_dt) if mm_dt != in_dt else ap

    # Raw PSUM banks, manually rotated (mi parity picks the half). Raw
    # tensors get full RAW/WAR/WAW tracking from TileContext's shadow
    # memory but none of the pool's slot-release waits.
    ps_banks = [
        nc.alloc_psum_tensor(f"psb{k}", [P, s_chunk], mybir.dt.float32).ap()
        for k in range(2 * b)
    ]

    with tile.TileContext(nc) as tc:
        with (
            tc.tile_pool(name="xpool", bufs=1) as xpool,
            tc.tile_pool(name="wpool", bufs=2) as wpool,
            tc.tile_pool(name="bpool", bufs=1) as bpool,
            tc.tile_pool(name="opool", bufs=4) as opool,
        ):
            bias_t = bpool.tile([P, n_mi], f32, tag="bias")
            nc.sync.dma_start(out=bias_t, in_=bias_d)

            # Resident x tiles: one [128, s_in] tile per (batch, k-tile).
            x_t = {}
            for bi in range(b):
                for ki in range(n_ki):
                    t_ = xpool.tile([P, s_in], mm_dt, tag=f"x{bi}_{ki}")
                    nc.sync.dma_start(
                        out=t_,
                        in_=dma_src(x_d[bi, ki * P : (ki + 1) * P, :]),
                    )
                    x_t[bi, ki] = t_

            for rep in range(reps):
              for mi in range(n_mi):
                psums = [
                    ps_banks[((rep * n_mi + mi) % 2) * b + bi]
                    for bi in range(b)
                ]
                for c in range(n_wchunks):
                    w_t = wpool.tile([P, wchunk_cols], mm_dt, tag="w")
                    # SWDGE (gpsimd): keeps weight traffic off the HWDGE
                    # queues used by x/out.
                    nc.gpsimd.dma_start(
                        out=w_t,
                        in_=dma_src(
                            w_d[
                                mi, :,
                                c * ki_per_wchunk : (c + 1) * ki_per_wchunk,
                                :, :,
                            ]
                        ),
                    )
                    for kic in range(ki_per_wchunk):
                        ki = c * ki_per_wchunk + kic
                        for t in range(ktaps):
                            col0 = (kic * ktaps + t) * P
                            lhsT = w_t[:, col0 : col0 + P]
                            first = ki == 0 and t == 0
                            last = ki == n_ki - 1 and t == ktaps - 1
                            for bi in range(b):
                                rhs = x_t[bi, ki][:, t : t + s_chunk]
                                nc.tensor.matmul(
                                    psums[bi], lhsT, rhs, start=first, stop=last
                                )
                for bi in range(b):
                    o_t = opool.tile([P, s_chunk], f32, tag="o")
                    nc.scalar.activation(
                        o_t, psums[bi], silu, bias=bias_t[:, mi : mi + 1]
                    )
                    nc.sync.dma_start(
                        out=out_d[bi, mi * P : (mi + 1) * P, :], in_=o_t
                    )
    nc.compile()
    return nc


def prep_weight(weight, n_mi, n_ki, ktaps):
    # (C_OUT, C_IN, K) -> (mi, p, ki, t, f) with o=(mi,f), i=(ki,p)
    w = weight.reshape(n_mi, P, n_ki, P, ktaps)  # (mi, f, ki, p, t)
    return np.ascontiguousarray(w.transpose(0, 3, 2, 4, 1))


def host_in_dtype(mm_dtype=None):
    mm_dtype = mm_dtype or MM_DTYPE
    if mm_dtype == "f16":
        return np.float16
    return np.float32


def kernel(x, weight, bias):
    x = np.asarray(x, dtype=np.float32)
    weight = np.asarray(weight, dtype=np.float32)
    bias = np.asarray(bias, dtype=np.float32)

    n_ki = C_IN // P
    n_mi = C_OUT // P
    hdt = host_in_dtype()

    xp = np.pad(x, ((0, 0), (0, 0), (HALO, 0)))  # (B, C_IN, S+3)
    w3 = prep_weight(weight, n_mi, n_ki, KTAPS).astype(hdt)
    bias2 = np.ascontiguousarray(bias.reshape(n_mi, P).T)  # (P, n_mi)

    nc = build_conv_nc(B, n_ki, n_mi, KTAPS, S_CHUNK, ki_per_wchunk=8)

    in_maps = []
    for c in range(N_CORES):
        xc = np.ascontiguousarray(
            xp[:, :, c * S_CHUNK : c * S_CHUNK + S_CHUNK + HALO]
        ).astype(hdt)
        in_maps.append({"x": xc, "w": w3, "bias": bias2})

    global LAST_RESULT
    res = run_bass_kernel_spmd(
        nc, in_maps, core_ids=list(range(N_CORES)), trace=PROFILE
    )
    LAST_RESULT = res
    out = np.concatenate([r["out"] for r in res.results], axis=2)
    return out


PROFILE = False
LAST_RESULT = None


# revision 20
# speedup vs baseline: 1.0236x; 1.0236x over previous
"""Causal Conv1d (K=4) + bias + silu for TRN2, sharded over 8 NeuronCores.

Reference op: x (B=4, C_IN=2048, S=4096) fp32, weight (C_OUT=2048, C_IN, 4),
bias (C_OUT,);  out = silu(causal_conv1d(x, weight) + bias).

Sharding: data-parallel over sequence. Core c computes out[:, :, c*512:(c+1)*512]
from x[:, :, c*512-3 : c*512+512] (zero-padded left halo), full weight/bias.

Per-core compute: the conv is 16 k-tiles x 4 taps = 64 accumulating
128x128x512 matmuls per (batch, m-tile) PSUM group. Tap t uses a shifted
free-dim view of the resident x tile - no shifted copies are materialized.
Matmuls accumulate in fp32 PSUM. Weights are pre-transposed on the host to
(mi, p, ki, t, f) so each per-m-tile weight chunk is one DMA with
long-contiguous per-partition lines. The 4 batches accumulate into 4 PSUM
banks in parallel so each weight tile's uses are back-to-back and the next
m-tile (4 more banks) overlaps eviction (silu+bias on ScalarE) and store.
"""

import numpy as np

import concourse.bacc as bacc
import concourse.bass as bass
import concourse.mybir as mybir
import concourse.tile as tile
from concourse.bass_utils import run_bass_kernel_spmd

P = 128

# Problem constants (hardcoded per harness contract).
B = 4
C_IN = 2048
C_OUT = 2048
KTAPS = 4
S = 4096
N_CORES = 8
S_CHUNK = S // N_CORES          # 512
HALO = KTAPS - 1                # 3

# Matmul operand dtype: "f32r" (FP22-truncated fp32, rel err ~1.6e-4) or
# "f16" (fp16 operands, rel err ~1e-3). Both measured ~1.2 ms/pass on HW;
# f32r kept for precision.
MM_DTYPE = "f32r"


def build_conv_nc(
    b, n_ki, n_mi, ktaps, s_chunk, ki_per_wchunk, act_fn=None, reps=1,
    mm_dtype=None,
):
    """Build the per-core Bass program.

    b:        batches
    n_ki:     C_IN / 128 contraction tiles
    n_mi:     C_OUT / 128 output tiles
    ktaps:    conv taps
    s_chunk:  output sequence columns per core
    ki_per_wchunk: k-tiles per weight DMA chunk
    """
    halo = ktaps - 1
    s_in = s_chunk + halo
    assert n_ki % ki_per_wchunk == 0
    n_wchunks = n_ki // ki_per_wchunk
    wchunk_cols = ki_per_wchunk * ktaps * P

    mm_dtype = mm_dtype or MM_DTYPE
    if mm_dtype == "f32r":
        in_dt = mybir.dt.float32      # dram dtype of x/w
        mm_dt = mybir.dt.float32r     # sbuf tile dtype fed to the PE
        cast_dma = False              # fp32->fp32r is a bitcast
    elif mm_dtype == "f16":
        in_dt = mybir.dt.float16      # host pre-converts x/w to fp16
        mm_dt = mybir.dt.float16
        cast_dma = False
    else:
        raise ValueError(mm_dtype)

    # Bacc (not raw Bass): its compile() splits multi-wait instructions into
    # event-semaphore sequences and moves matmul waits onto ldweights —
    # without it, walrus rejects any instruction carrying >1 sync wait.
    nc = bacc.Bacc("TRN2", target_bir_lowering=False, debug=False)

    x_d = nc.dram_tensor(
        "x", [b, n_ki * P, s_in], in_dt, kind="ExternalInput"
    ).ap()
    # weight pre-layout: (mi, p, ki, t, f) = W[mi*128+f, ki*128+p, t]
    w_d = nc.dram_tensor(
        "w", [n_mi, P, n_ki, ktaps, P], in_dt, kind="ExternalInput"
    ).ap()
    # bias pre-layout: [p, mi] = bias[mi*128+p]
    bias_d = nc.dram_tensor(
        "bias", [P, n_mi], mybir.dt.float32, kind="ExternalInput"
    ).ap()
    out_d = nc.dram_tensor(
        "out", [b, n_mi * P, s_chunk], mybir.dt.float32, kind="ExternalOutput"
    ).ap()

    f32 = mybir.dt.float32
    silu = act_fn if act_fn is not None else mybir.ActivationFunctionType.Silu

    def dma_src(ap):
        # fp32r tiles must be *written* as fp32r (BIR verifier rule);
        # the DRAM side carries the same bits, so bitcast the source.
        return ap.bitcast(mm_dt) if mm_dt != in_dt else ap

    # Raw PSUM banks, manually rotated (mi parity picks the half). Raw
    # tensors get full RAW/WAR/WAW tracking from TileContext's shadow
    # memory but none of the pool's slot-release waits.
    ps_banks = [
        nc.alloc_psum_tensor(f"psb{k}", [P, s_chunk], mybir.dt.float32).ap()
        for k in range(2 * b)
    ]

    with tile.TileContext(nc) as tc:
        with (
            tc.tile_pool(name="xpool", bufs=1) as xpool,
            tc.tile_pool(name="wpool", bufs=2) as wpool,
            tc.tile_pool(name="bpool", bufs=1) as bpool,
            tc.tile_pool(name="opool", bufs=4) as opool,
        ):
            bias_t = bpool.tile([P, n_mi], f32, tag="bias")
            nc.sync.dma_start(out=bias_t, in_=bias_d)

            # Resident x tiles: one [128, s_in] tile per (batch, k-tile).
            x_t = {}
            for bi in range(b):
                for ki in range(n_ki):
                    t_ = xpool.tile([P, s_in], mm_dt, tag=f"x{bi}_{ki}")
                    nc.sync.dma_start(
                        out=t_,
                        in_=dma_src(x_d[bi, ki * P : (ki + 1) * P, :]),
                    )
                    x_t[bi, ki] = t_

            for rep in range(reps):
              for mi in range(n_mi):
                psums = [
                    ps_banks[((rep * n_mi + mi) % 2) * b + bi]
                    for bi in range(b)
                ]
                for c in range(n_wchunks):
                    w_t = wpool.tile([P, wchunk_cols], mm_dt, tag="w")
                    # SWDGE (gpsimd): keeps weight traffic off the HWDGE
                    # queues used by x/out.
                    nc.gpsimd.dma_start(
                        out=w_t,
                        in_=dma_src(
                            w_d[
                                mi, :,
                                c * ki_per_wchunk : (c + 1) * ki_per_wchunk,
                                :, :,
                            ]
                        ),
                    )
                    for kic in range(ki_per_wchunk):
                        ki = c * ki_per_wchunk + kic
                        for t in range(ktaps):
                            col0 = (kic * ktaps + t) * P
                            lhsT = w_t[:, col0 : col0 + P]
                            first = ki == 0 and t == 0
                            last = ki == n_ki - 1 and t == ktaps - 1
                            for bi in range(b):
                                rhs = x_t[bi, ki][:, t : t + s_chunk]
                                nc.tensor.matmul(
                                    psums[bi], lhsT, rhs, start=first, stop=last
                                )
                for bi in range(b):
                    o_t = opool.tile([P, s_chunk], f32, tag="o")
                    nc.scalar.activation(
                        o_t, psums[bi], silu, bias=bias_t[:, mi : mi + 1]
                    )
                    nc.sync.dma_start(
                        out=out_d[bi, mi * P : (mi + 1) * P, :], in_=o_t
                    )
    nc.compile()
    return nc


def prep_weight(weight, n_mi, n_ki, ktaps):
    # (C_OUT, C_IN, K) -> (mi, p, ki, t, f) with o=(mi,f), i=(ki,p)
    w = weight.reshape(n_mi, P, n_ki, P, ktaps)  # (mi, f, ki, p, t)
    return np.ascontiguousarray(w.transpose(0, 3, 2, 4, 1))


def host_in_dtype(mm_dtype=None):
    mm_dtype = mm_dtype or MM_DTYPE
    if mm_dtype == "f16":
        return np.float16
    return np.float32


def kernel(x, weight, bias):
    x = np.asarray(x, dtype=np.float32)
    weight = np.asarray(weight, dtype=np.float32)
    bias = np.asarray(bias, dtype=np.float32)

    n_ki = C_IN // P
    n_mi = C_OUT // P
    hdt = host_in_dtype()

    xp = np.pad(x, ((0, 0), (0, 0), (HALO, 0)))  # (B, C_IN, S+3)
    w3 = prep_weight(weight, n_mi, n_ki, KTAPS).astype(hdt)
    bias2 = np.ascontiguousarray(bias.reshape(n_mi, P).T)  # (P, n_mi)

    nc = build_conv_nc(B, n_ki, n_mi, KTAPS, S_CHUNK, ki_per_wchunk=8)

    in_maps = []
    for c in range(N_CORES):
        xc = np.ascontiguousarray(
            xp[:, :, c * S_CHUNK : c * S_CHUNK + S_CHUNK + HALO]
        ).astype(hdt)
        in_maps.append({"x": xc, "w": w3, "bias": bias2})

    global LAST_RESULT
    res = run_bass_kernel_spmd(
        nc, in_maps, core_ids=list(range(N_CORES)), trace=PROFILE
    )
    LAST_RESULT = res
    out = np.concatenate([r["out"] for r in res.results], axis=2)
    return out


PROFILE = False
LAST_RESULT = None
